# revision 34
# baseline (speedup 1.0000x reference)
"""GQA causal prefill attention on 8 TRN2 NeuronCores.

Schedule v2: the ACT (scalar) engine is the critical resource — every causal
score element must pass through one Exp ACTIVATE at 1 elem/cycle/lane, a
~58us/core floor. This version minimizes everything else on ACT:

- exp windows are 12-tile PSUM units that span j-block boundaries (48-49
  ACTIVATEs total vs 69), double-buffered so the PE fills one unit while
  ACT drains the other;
- no DMAs on the scalar queue (kT on sync, qT0 on vector, v+qT1-3 on gpsimd);
- output tiles are written into 4-tile SBUF strips and DMA'd per strip
  (16 output DMAs instead of 64);
- PV chains are interleaved between exp windows via a ready-queue budget so
  the PE stays busy while ACT streams.

Host side unchanged: q pre-transposed to qT (4, 128, 2048) bf16, k to kT
(128, 2048) bf16, v (2048, 128) bf16; output bf16, upcast on host."""

import sys
import functools

import numpy as np

if "/opt/trn_rl_repo" not in sys.path:
    sys.path.insert(0, "/opt/trn_rl_repo")

T = 2048
H_TOTAL = 32
N_CORES = 8
H = H_TOTAL // N_CORES  # 4 q heads per core
D = 128
P = 128
NT = T // P  # 16 token tiles
SCALE = 0.08838834764831845

# cumulative tile offsets of causal blocks: block j holds tiles i=j..15
CEOFF = [0] * (NT + 1)
for _j in range(NT):
    CEOFF[_j + 1] = CEOFF[_j] + (NT - _j)
NTILES = CEOFF[NT]  # 136
E_COLS = NTILES * P  # 17408

ST_TILES = 12  # PSUM exp unit size (3 banks; 2 units + 2 PV banks = 8)


def _block_of(g):
    for j in range(NT):
        if CEOFF[j + 1] > g:
            return j
    raise AssertionError


def _windows(lo, hi, w=ST_TILES):
    """Ascending w-tile windows covering tile range [lo, hi)."""
    out = []
    a = lo
    while a < hi:
        b = min(a + w, hi)
        out.append((a, b))
        a = b
    return out


def _build_body(tc, nc, q_d, k_d, v_d, o_d, ctx):
    from collections import deque

    import concourse.mybir as mybir
    from concourse.masks import make_identity, make_upper_triangular

    f32 = mybir.dt.float32
    bf16 = mybir.dt.bfloat16

    const = ctx.enter_context(tc.tile_pool(name="const", bufs=1))
    qtp = ctx.enter_context(tc.tile_pool(name="qT", bufs=4))
    ep = ctx.enter_context(tc.tile_pool(name="eT", bufs=4))
    outp = ctx.enter_context(tc.tile_pool(name="outt", bufs=3))
    recp = ctx.enter_context(tc.tile_pool(name="rec", bufs=4))

    st_pool = ctx.enter_context(tc.tile_pool(name="st", bufs=2, space="PSUM"))
    sm_pool = ctx.enter_context(tc.tile_pool(name="smp", bufs=2, space="PSUM"))

    # out is (4_g, H, P, 4_u*D) on device: per-partition runs of 1KiB
    # (host permutes back to (T, H, D))
    o_view = o_d.rearrange("g h p e -> p g h e")

    kT = const.tile([P, NT, P], bf16, tag="kT")        # [d, j, s]
    v_aug = const.tile([P, NT, D + 1], bf16, tag="vaug")
    qT = [
        qtp.tile([P, NT, P], bf16, tag="qT", name=f"qT{h}") for h in range(H)
    ]

    k_view = k_d.rearrange("d (j p) -> d j p", p=P)
    q_view = q_d.rearrange("h d (i p) -> h d i p", p=P)
    # v pre-packed on host to [p, j, d+1] with the ones column baked in
    v_view = v_d.rearrange("p (j e) -> p j e", e=D + 1)
    # kT/qT0 DMAs first on sync/scalar queues: high half first (covers all of
    # head 0's phase-1 blocks 8-15 in one transfer), then the low half
    for (s, n) in ((8, 8), (0, 8)):
        nc.sync.dma_start(kT[:, s:s + n, :], k_view[:, s:s + n, :])
        nc.scalar.dma_start(qT[0][:, s:s + n, :], q_view[0, :, s:s + n, :])

    # consts on the gpsimd engine BEFORE its DMA issues: the PE warmups and
    # the first windows gate on the identity
    identity = const.tile([P, P], bf16, tag="ident")
    make_identity(nc, identity)
    utri = const.tile([P, P], bf16, tag="utri")
    make_upper_triangular(nc, utri, val=1.0, diag=True)

    nc.gpsimd.dma_start(v_aug, v_view)
    for h in range(1, H):
        nc.gpsimd.dma_start(qT[h], q_view[h])

    warm_sb = recp.tile([P, 1], f32, tag="rec", name="warm")

    # PV accumulators: 3 slots of 129 cols per PSUM bank, 2 banks = 6
    # in-flight chains (vs 2 with one-tile-per-bank) — the chain-start
    # WAR on the slot reaches 6 chains back instead of 2
    pv_slots = [
        sm_pool.tile([P, 3 * (D + 1)], f32, tag="sm", name=f"pvs{u}")
        for u in range(2)
    ]
    chain_ctr = [0]

    # ACT table prewarm: pays the exp table load before the first real window
    nc.scalar.activation(
        out=warm_sb, in_=identity[:, 0:1],
        func=mybir.ActivationFunctionType.Exp,
    )

    # PE warmup: real matmuls on the identity ramp the PE p-state clock
    # while the first input chunks land.
    warm_mm = sm_pool.tile([P, P], f32, tag="sm", name="warmmm")
    for _ in range(14):
        nc.tensor.matmul(warm_mm, lhsT=identity, rhs=identity,
                         start=True, stop=True)

    strip = {}

    def finish_chain(eT, h, i, pv):
        rec = recp.tile([P, 1], f32, tag="rec")
        nc.vector.reciprocal(rec, pv[:, D:D + 1])
        u = i % 4
        if u == 0:
            strip[h] = outp.tile([P, 4 * P], bf16, tag="outt", name="ostrip")
        nc.vector.tensor_scalar_mul(
            strip[h][:, u * P:(u + 1) * P], pv[:, 0:D], rec
        )
        if u == 3:
            nc.sync.dma_start(o_view[:, i // 4, h, :], strip[h])

    ready = deque()  # (unlock_window, eT, h, i)
    wcount = [0]

    def emit_chain(e2, h2, i2):
        s = chain_ctr[0] % 6
        chain_ctr[0] += 1
        pv = pv_slots[s // 3][:, (s % 3) * (D + 1):(s % 3 + 1) * (D + 1)]
        for j in range(i2 + 1):
            c0 = (CEOFF[j] + (i2 - j)) * P
            nc.tensor.matmul(
                pv,
                lhsT=e2[:, c0:c0 + P],
                rhs=v_aug[:, j, :],
                start=(j == 0),
                stop=(j == i2),
            )
        finish_chain(e2, h2, i2, pv)

    def pop_ready(budget, force=False):
        while ready:
            w0, e2, h2, i2 = ready[0]
            size = i2 + 1
            if not force and size > budget and budget < 16:
                break
            ready.popleft()
            emit_chain(e2, h2, i2)
            budget -= size
            if budget <= 0 and not force:
                break

    def do_window(h, eT, a, b, warm=0):
        """Score matmuls for eT tiles [a, b) + one exp ACTIVATE."""
        stu = st_pool.tile([P, ST_TILES * P], f32, tag="st")
        g = a
        while g < b:
            j = _block_of(g)
            i = j + (g - CEOFF[j])
            m = min(4 - ((g - a) % 4), CEOFF[j + 1] - g, b - g)
            nc.tensor.matmul(
                stu[:, (g - a) * P:(g - a + m) * P],
                lhsT=kT[:, j, :],
                rhs=qT[h][:, i:i + m, :],
                start=True,
                stop=True,
            )
            g += m
        for _ in range(warm):
            nc.tensor.matmul(warm_mm, lhsT=identity, rhs=identity,
                             start=True, stop=True)
        nc.scalar.activation(
            out=eT[:, a * P:b * P],
            in_=stu[:, 0:(b - a) * P],
            func=mybir.ActivationFunctionType.Exp,
            scale=SCALE,
        )
        wcount[0] += 1

    def finish_blocks(h, eT, a, b, ready_rows):
        """Mask diagonal tile of every block completed by window [a, b)."""
        for j in range(NT):
            if a < CEOFF[j + 1] <= b:
                off = CEOFF[j] * P
                nc.vector.tensor_tensor(
                    eT[:, off:off + P],
                    eT[:, off:off + P],
                    utri,
                    mybir.AluOpType.mult,
                )
                if ready_rows:
                    ready.append((wcount[0], eT, h, j))

    # head 0: high-j region first (kT high chunk arrives first), then low-j.
    h0_phase1 = [(CEOFF[12], NTILES), (CEOFF[10], CEOFF[12]),
                 (CEOFF[9], CEOFF[10]), (CEOFF[8], CEOFF[9])]
    h0_phase2 = _windows(0, CEOFF[8])

    for h in range(H):
        eT = ep.tile([P, E_COLS], bf16, tag="eT")
        if h == 0:
            for wi, (a, b) in enumerate(h0_phase1):
                # no PV work exists yet; keep the PE clock warm instead
                do_window(0, eT, a, b, warm=6 if wi < 3 else 0)
                finish_blocks(0, eT, a, b, ready_rows=False)
            nw2 = len(h0_phase2)
            for wi, (a, b) in enumerate(h0_phase2):
                pop_ready(2 if wi == nw2 - 1 else 14)
                do_window(0, eT, a, b)
                finish_blocks(0, eT, a, b, ready_rows=True)
            # rows 8-15 of head 0 become ready once blocks 0-7 are done
            for i in range(8, NT):
                ready.append((wcount[0], eT, 0, i))
        else:
            wins = _windows(0, NTILES)
            nw = len(wins)
            for wi, (a, b) in enumerate(wins):
                # around head boundaries the adjacent exp is a short 4-tile
                # window: a full burst there delays scores and stalls ACT
                if wi == 0 or wi == nw - 1:
                    budget = 2
                else:
                    budget = 18 if h == H - 1 else 14
                pop_ready(budget)
                do_window(h, eT, a, b)
                finish_blocks(h, eT, a, b, ready_rows=True)
    pop_ready(0, force=True)


@functools.lru_cache(maxsize=1)
def _build():
    import concourse.tile as tile
    import concourse.mybir as mybir
    from concourse import bacc
    from contextlib import ExitStack

    nc = bacc.Bacc(
        "TRN2",
        target_bir_lowering=False,
        debug=False,
        num_devices=N_CORES,
    )
    bf16 = mybir.dt.bfloat16
    q_d = nc.dram_tensor("q", (H, D, T), bf16, kind="ExternalInput").ap()
    k_d = nc.dram_tensor("k", (D, T), bf16, kind="ExternalInput").ap()
    v_d = nc.dram_tensor("v", (P, NT * (D + 1)), bf16, kind="ExternalInput").ap()
    o_d = nc.dram_tensor("out", (4, H, P, 4 * D), bf16, kind="ExternalOutput").ap()

    with tile.TileContext(nc) as tc:
        with ExitStack() as ctx:
            _build_body(tc, nc, q_d, k_d, v_d, o_d, ctx)
    nc.compile()
    return nc


def _in_maps(q, k, v):
    import ml_dtypes

    bf16 = ml_dtypes.bfloat16
    q = np.asarray(q, dtype=np.float32)
    k = np.asarray(k, dtype=np.float32)
    v = np.asarray(v, dtype=np.float32)
    maps = []
    ones = np.ones((T, 1), dtype=np.float32)
    for c in range(N_CORES):
        qt = np.ascontiguousarray(
            q[:, H * c:H * c + H, :].transpose(1, 2, 0)
        ).astype(bf16)
        kt = np.ascontiguousarray(k[:, c, :].T).astype(bf16)
        # v packed as [p, j, d+1] with ones column baked in (1 elem/row sums)
        va = np.concatenate([v[:, c, :], ones], axis=1)  # (T, 129)
        vc = np.ascontiguousarray(
            va.reshape(NT, P, D + 1).transpose(1, 0, 2)
        ).reshape(P, NT * (D + 1)).astype(bf16)
        maps.append({"q": qt, "k": kt, "v": vc})
    return maps


def kernel(q, k, v, _trace=False):
    from concourse.bass_utils import run_bass_kernel_spmd

    nc = _build()
    res = run_bass_kernel_spmd(
        nc, _in_maps(q, k, v), core_ids=list(range(N_CORES)), trace=_trace
    )
    out = np.empty((T, H_TOTAL, D), dtype=np.float32)
    for c in range(N_CORES):
        dev = np.asarray(res.results[c]["out"], dtype=np.float32)
        dev = dev.reshape(4, H, P, 4, D).transpose(0, 3, 2, 1, 4)
        out[:, H * c:H * c + H, :] = dev.reshape(T, H, D)
    if _trace:
        return out, res
    return out


# revision 39
# speedup vs baseline: 1.0459x; 1.0459x over previous
"""GQA causal prefill attention on 8 TRN2 NeuronCores.

Schedule v2: the ACT (scalar) engine is the critical resource — every causal
score element must pass through one Exp ACTIVATE at 1 elem/cycle/lane, a
~58us/core floor. This version minimizes everything else on ACT:

- exp windows are 12-tile PSUM units that span j-block boundaries (48-49
  ACTIVATEs total vs 69), double-buffered so the PE fills one unit while
  ACT drains the other;
- no DMAs on the scalar queue (kT on sync, qT0 on vector, v+qT1-3 on gpsimd);
- output tiles are written into 4-tile SBUF strips and DMA'd per strip
  (16 output DMAs instead of 64);
- PV chains are interleaved between exp windows via a ready-queue budget so
  the PE stays busy while ACT streams.

Host side unchanged: q pre-transposed to qT (4, 128, 2048) bf16, k to kT
(128, 2048) bf16, v (2048, 128) bf16; output bf16, upcast on host."""

import sys
import functools

import numpy as np

if "/opt/trn_rl_repo" not in sys.path:
    sys.path.insert(0, "/opt/trn_rl_repo")

T = 2048
H_TOTAL = 32
N_CORES = 8
H = H_TOTAL // N_CORES  # 4 q heads per core
D = 128
P = 128
NT = T // P  # 16 token tiles
SCALE = 0.08838834764831845

# cumulative tile offsets of causal blocks: block j holds tiles i=j..15
CEOFF = [0] * (NT + 1)
for _j in range(NT):
    CEOFF[_j + 1] = CEOFF[_j] + (NT - _j)
NTILES = CEOFF[NT]  # 136
E_COLS = NTILES * P  # 17408

ST_TILES = 12  # PSUM exp unit size (3 banks; 2 units + 2 PV banks = 8)


def _block_of(g):
    for j in range(NT):
        if CEOFF[j + 1] > g:
            return j
    raise AssertionError


def _windows(lo, hi, w=ST_TILES):
    """Ascending w-tile windows covering tile range [lo, hi)."""
    out = []
    a = lo
    while a < hi:
        b = min(a + w, hi)
        out.append((a, b))
        a = b
    return out


def _build_body(tc, nc, q_d, k_d, v_d, o_d, ctx):
    from collections import deque

    import concourse.mybir as mybir
    from concourse.masks import make_identity, make_upper_triangular

    f32 = mybir.dt.float32
    bf16 = mybir.dt.bfloat16

    const = ctx.enter_context(tc.tile_pool(name="const", bufs=1))
    qtp = ctx.enter_context(tc.tile_pool(name="qT", bufs=4))
    ep = ctx.enter_context(tc.tile_pool(name="eT", bufs=4))
    outp = ctx.enter_context(tc.tile_pool(name="outt", bufs=3))
    recp = ctx.enter_context(tc.tile_pool(name="rec", bufs=4))

    st_pool = ctx.enter_context(tc.tile_pool(name="st", bufs=2, space="PSUM"))
    sm_pool = ctx.enter_context(tc.tile_pool(name="smp", bufs=2, space="PSUM"))

    # out is (4_g, H, P, 4_u*D) on device: per-partition runs of 1KiB
    # (host permutes back to (T, H, D))
    o_view = o_d.rearrange("g h p e -> p g h e")

    kT = const.tile([P, NT, P], bf16, tag="kT")        # [d, j, s]
    v_aug = const.tile([P, NT, D + 1], bf16, tag="vaug")
    qT = [
        qtp.tile([P, NT, P], bf16, tag="qT", name=f"qT{h}") for h in range(H)
    ]

    k_view = k_d.rearrange("d (j p) -> d j p", p=P)
    q_view = q_d.rearrange("h d (i p) -> h d i p", p=P)
    # v pre-packed on host to [p, j, d+1] with the ones column baked in
    v_view = v_d.rearrange("p (j e) -> p j e", e=D + 1)
    # kT/qT0 DMAs first on sync/scalar queues: high chunk first (phase-1
    # blocks 8-13; 14-15 ride the gpsimd queue), then the low half
    for (s, n) in ((8, 6), (0, 8)):
        nc.sync.dma_start(kT[:, s:s + n, :], k_view[:, s:s + n, :])
        nc.scalar.dma_start(qT[0][:, s:s + n, :], q_view[0, :, s:s + n, :])

    # consts on the gpsimd engine BEFORE its DMA issues: the PE warmups and
    # the first windows gate on the identity
    identity = const.tile([P, P], bf16, tag="ident")
    make_identity(nc, identity)
    utri = const.tile([P, P], bf16, tag="utri")
    make_upper_triangular(nc, utri, val=1.0, diag=True)

    # tiny head-start chunk on the otherwise-idle gpsimd queue: blocks 14-15
    # (first phase-1 window) land ~2us before the sync/scalar halves
    nc.gpsimd.dma_start(kT[:, 14:16, :], k_view[:, 14:16, :])
    nc.gpsimd.dma_start(qT[0][:, 14:16, :], q_view[0, :, 14:16, :])
    nc.gpsimd.dma_start(v_aug, v_view)
    for h in range(1, H):
        nc.gpsimd.dma_start(qT[h], q_view[h])

    warm_sb = recp.tile([P, 1], f32, tag="rec", name="warm")



    # ACT table prewarm: pays the exp table load before the first real window
    nc.scalar.activation(
        out=warm_sb, in_=identity[:, 0:1],
        func=mybir.ActivationFunctionType.Exp,
    )

    # PE warmup: real matmuls on the identity ramp the PE p-state clock
    # while the first input chunks land.
    warm_mm = sm_pool.tile([P, P], f32, tag="sm", name="warmmm")
    for _ in range(14):
        nc.tensor.matmul(warm_mm, lhsT=identity, rhs=identity,
                         start=True, stop=True)

    strip = {}

    def finish_chain(eT, h, i, pv):
        rec = recp.tile([P, 1], f32, tag="rec")
        nc.vector.reciprocal(rec, pv[:, D:D + 1])
        u = i % 4
        if u == 0:
            strip[h] = outp.tile([P, 4 * P], bf16, tag="outt", name="ostrip")
        nc.vector.tensor_scalar_mul(
            strip[h][:, u * P:(u + 1) * P], pv[:, 0:D], rec
        )
        if u == 3:
            nc.sync.dma_start(o_view[:, i // 4, h, :], strip[h])

    ready = deque()  # (unlock_window, eT, h, i)
    wcount = [0]

    def emit_chain(e2, h2, i2):
        pv = sm_pool.tile([P, P + 1], f32, tag="sm", name="pv")
        for j in range(i2 + 1):
            c0 = (CEOFF[j] + (i2 - j)) * P
            nc.tensor.matmul(
                pv,
                lhsT=e2[:, c0:c0 + P],
                rhs=v_aug[:, j, :],
                start=(j == 0),
                stop=(j == i2),
            )
        finish_chain(e2, h2, i2, pv)

    def pop_ready(budget, force=False):
        while ready:
            w0, e2, h2, i2 = ready[0]
            size = i2 + 1
            if not force and size > budget and budget < 16:
                break
            ready.popleft()
            emit_chain(e2, h2, i2)
            budget -= size
            if budget <= 0 and not force:
                break

    def do_window(h, eT, a, b, warm=0):
        """Score matmuls for eT tiles [a, b) + one exp ACTIVATE."""
        stu = st_pool.tile([P, ST_TILES * P], f32, tag="st")
        g = a
        while g < b:
            j = _block_of(g)
            i = j + (g - CEOFF[j])
            m = min(4 - ((g - a) % 4), CEOFF[j + 1] - g, b - g)
            nc.tensor.matmul(
                stu[:, (g - a) * P:(g - a + m) * P],
                lhsT=kT[:, j, :],
                rhs=qT[h][:, i:i + m, :],
                start=True,
                stop=True,
            )
            g += m
        for _ in range(warm):
            nc.tensor.matmul(warm_mm, lhsT=identity, rhs=identity,
                             start=True, stop=True)
        nc.scalar.activation(
            out=eT[:, a * P:b * P],
            in_=stu[:, 0:(b - a) * P],
            func=mybir.ActivationFunctionType.Exp,
            scale=SCALE,
        )
        wcount[0] += 1

    def finish_blocks(h, eT, a, b, ready_rows):
        """Mask diagonal tile of every block completed by window [a, b)."""
        for j in range(NT):
            if a < CEOFF[j + 1] <= b:
                off = CEOFF[j] * P
                nc.vector.tensor_tensor(
                    eT[:, off:off + P],
                    eT[:, off:off + P],
                    utri,
                    mybir.AluOpType.mult,
                )
                if ready_rows:
                    ready.append((wcount[0], eT, h, j))

    # head 0: high-j region first (kT high chunk arrives first), then low-j.
    # blocks 14-15 alone first: their 32KiB chunk lands earliest
    h0_phase1 = [(CEOFF[14], NTILES), (CEOFF[12], CEOFF[14]),
                 (CEOFF[10], CEOFF[12]),
                 (CEOFF[9], CEOFF[10]), (CEOFF[8], CEOFF[9])]
    h0_phase2 = _windows(0, CEOFF[8])

    for h in range(H):
        eT = ep.tile([P, E_COLS], bf16, tag="eT")
        if h == 0:
            for wi, (a, b) in enumerate(h0_phase1):
                # no PV work exists yet; keep the PE clock warm instead
                do_window(0, eT, a, b, warm=6 if wi < 3 else 0)
                finish_blocks(0, eT, a, b, ready_rows=False)
            nw2 = len(h0_phase2)
            for wi, (a, b) in enumerate(h0_phase2):
                pop_ready(2 if wi == nw2 - 1 else 14)
                do_window(0, eT, a, b)
                finish_blocks(0, eT, a, b, ready_rows=True)
            # rows 8-15 of head 0 become ready once blocks 0-7 are done
            for i in range(8, NT):
                ready.append((wcount[0], eT, 0, i))
        else:
            wins = _windows(0, NTILES)
            nw = len(wins)
            for wi, (a, b) in enumerate(wins):
                # around head boundaries the adjacent exp is a short 4-tile
                # window: a full burst there delays scores and stalls ACT
                if wi == 0 or wi == nw - 1:
                    budget = 2
                else:
                    budget = 18 if h == H - 1 else 14
                pop_ready(budget)
                do_window(h, eT, a, b)
                finish_blocks(h, eT, a, b, ready_rows=True)
    pop_ready(0, force=True)


@functools.lru_cache(maxsize=1)
def _build():
    import concourse.tile as tile
    import concourse.mybir as mybir
    from concourse import bacc
    from contextlib import ExitStack

    nc = bacc.Bacc(
        "TRN2",
        target_bir_lowering=False,
        debug=False,
        num_devices=N_CORES,
    )
    bf16 = mybir.dt.bfloat16
    q_d = nc.dram_tensor("q", (H, D, T), bf16, kind="ExternalInput").ap()
    k_d = nc.dram_tensor("k", (D, T), bf16, kind="ExternalInput").ap()
    v_d = nc.dram_tensor("v", (P, NT * (D + 1)), bf16, kind="ExternalInput").ap()
    o_d = nc.dram_tensor("out", (4, H, P, 4 * D), bf16, kind="ExternalOutput").ap()

    with tile.TileContext(nc) as tc:
        with ExitStack() as ctx:
            _build_body(tc, nc, q_d, k_d, v_d, o_d, ctx)
    nc.compile()
    return nc


def _in_maps(q, k, v):
    import ml_dtypes

    bf16 = ml_dtypes.bfloat16
    q = np.asarray(q, dtype=np.float32)
    k = np.asarray(k, dtype=np.float32)
    v = np.asarray(v, dtype=np.float32)
    maps = []
    ones = np.ones((T, 1), dtype=np.float32)
    for c in range(N_CORES):
        qt = np.ascontiguousarray(
            q[:, H * c:H * c + H, :].transpose(1, 2, 0)
        ).astype(bf16)
        kt = np.ascontiguousarray(k[:, c, :].T).astype(bf16)
        # v packed as [p, j, d+1] with ones column baked in (1 elem/row sums)
        va = np.concatenate([v[:, c, :], ones], axis=1)  # (T, 129)
        vc = np.ascontiguousarray(
            va.reshape(NT, P, D + 1).transpose(1, 0, 2)
        ).reshape(P, NT * (D + 1)).astype(bf16)
        maps.append({"q": qt, "k": kt, "v": vc})
    return maps


def kernel(q, k, v, _trace=False):
    from concourse.bass_utils import run_bass_kernel_spmd

    nc = _build()
    res = run_bass_kernel_spmd(
        nc, _in_maps(q, k, v), core_ids=list(range(N_CORES)), trace=_trace
    )
    out = np.empty((T, H_TOTAL, D), dtype=np.float32)
    for c in range(N_CORES):
        dev = np.asarray(res.results[c]["out"], dtype=np.float32)
        dev = dev.reshape(4, H, P, 4, D).transpose(0, 3, 2, 1, 4)
        out[:, H * c:H * c + H, :] = dev.reshape(T, H, D)
    if _trace:
        return out, res
    return out
